# revision 33
# baseline (speedup 1.0000x reference)
"""Trainium2 Bass kernel for nn_Attention_46901042872659.

Dense transformer attention block:
  qkv = BN(x @ qkv_w.T); split q,k,v per head; attn = softmax(q k^T * scale + bias);
  out = hardswish(attn @ v); y = BN(out @ proj_w.T)

Strategy: data-parallel over batch across 8 NeuronCores (8 batch elems each).
All matmuls in fp32r (full PE rate at moving-dim>=256, ~1e-4 accuracy).
Eval-mode BN is folded into the GEMM weights/bias on the host; the softmax
scale is folded into the q weights; the relative-position bias table is
gathered host-side into a dense [heads, j, i] matrix (constant wrt x).

Device program per core (single compiled module, run SPMD on 8 cores):
  Phase 1: qkv GEMM over all 2048 tokens.
    - q,k produced channel-major (q_T/k_T [64, tokens] per head) -> qk_scr DRAM
    - v produced token-major ([tokens, 3072]) -> v_scr DRAM
  Phase 2: per batch element b (8 of them):
    - S_T[j,i] = k_T^T q_T (contraction d=64), + bias (DVE), exp (ACT)
    - rowsum over j via ones-matmul; reciprocal; broadcast via rank-1 matmul
    - O_T[dv,i] = v^T @ expS_T (contraction j=256)
    - hardswish(O_T * recip) -> h_T (channel-major) feeds proj GEMM directly
    - y_T = Wp_T^T @ h_T (contraction d=3072) + BN fold -> out
"""
import numpy as np
import ml_dtypes
from contextlib import ExitStack

import concourse.bass as bass
import concourse.tile as tile
from concourse import bacc, mybir
from concourse.bass_utils import run_bass_kernel_spmd

# problem constants (hardcoded per contest contract)
B, SEQ, DIM = 64, 256, 768
HEADS, KD, DV = 12, 64, 256
H = 4608
DH = 3072
EPS = 1e-5
SCALE = KD ** -0.5
NCORES = 8
BPC = B // NCORES          # batch elems per core
T = BPC * SEQ              # tokens per core = 2048
F32 = mybir.dt.float32
F32R = mybir.dt.float32r
BF16 = mybir.dt.bfloat16
ADD = mybir.AluOpType.add
MULT = mybir.AluOpType.mult
MAX = mybir.AluOpType.max
MIN = mybir.AluOpType.min


def _phase1(tc, nc, x_t, wqk_t, wv_t, bqk, bv, qk_scr, v_scr):
    """qkv GEMM for all T tokens. x_T resident; weights resident."""
    with ExitStack() as ctx:
        res = ctx.enter_context(tc.tile_pool(name="p1res", bufs=1))
        st = ctx.enter_context(tc.tile_pool(name="p1st", bufs=2))
        ps = ctx.enter_context(tc.tile_pool(name="p1ps", bufs=1, space="PSUM"))

        x_sb = res.tile([128, 6, T], F32R)
        wqk_sb = res.tile([128, 6, 1536], F32R)
        wv_sb = res.tile([128, 6, DH], F32R)
        bqk_sb = res.tile([128, 12], F32)
        bvb_sb = res.tile([128, DH], BF16)
        for c in range(6):
            nc.sync.dma_start(x_sb[:, c, :], x_t.ap()[c])
            nc.sync.dma_start(wqk_sb[:, c, :], wqk_t.ap()[c])
        for c in range(6):
            nc.sync.dma_start(wv_sb[:, c, :], wv_t.ap()[c])
        nc.sync.dma_start(bqk_sb[:], bqk.ap())
        bv_ap = bv.ap()
        bv_bcast = bass.AP(tensor=bv_ap.tensor, offset=bv_ap.offset,
                           ap=[[0, 128]] + [list(p) for p in bv_ap.ap])
        nc.gpsimd.dma_start(bvb_sb[:], bv_bcast)

        # q,k part: out channel-major [co 128, t], lhsT = Wqk tile, rhs = x_T
        for cot in range(12):
            qkst = st.tile([128, T], F32R, name=f"qkst{cot}", tag="qkst")
            for tb in range(4):
                qps = ps.tile([128, 512], F32, name=f"qps{cot}_{tb}", tag="qps",
                              bufs=2)
                for c in range(6):
                    nc.tensor.matmul(
                        qps[:],
                        wqk_sb[:, c, cot * 128:(cot + 1) * 128],
                        x_sb[:, c, tb * 512:(tb + 1) * 512],
                        start=(c == 0), stop=(c == 5))
                nc.vector.tensor_scalar_add(
                    qkst[:, tb * 512:(tb + 1) * 512], qps[:], bqk_sb[:, cot:cot + 1])
            nc.sync.dma_start(qk_scr.ap()[cot], qkst[:])

        # v part: out token-major [t 128, co], lhsT = x_T tile (stationary), rhs = Wv
        for tt in range(16):
            vst = st.tile([128, DH], F32R, name=f"vst{tt}", tag="vst")
            vps_l = [ps.tile([128, 512], F32, name=f"vps{tt}_{cob}", tag=f"vps{cob}")
                     for cob in range(6)]
            for c in range(6):
                for cob in range(6):
                    nc.tensor.matmul(
                        vps_l[cob][:],
                        x_sb[:, c, tt * 128:(tt + 1) * 128],
                        wv_sb[:, c, cob * 512:(cob + 1) * 512],
                        start=(c == 0), stop=(c == 5))
            for cob in range(6):
                nc.vector.tensor_tensor(
                    vst[:, cob * 512:(cob + 1) * 512], vps_l[cob][:],
                    bvb_sb[:, cob * 512:(cob + 1) * 512], ADD)
            nc.sync.dma_start(v_scr.ap()[tt], vst[:])


def _phase2(tc, nc, wp_t, bsc, pg, pb, ones_c, qk_scr, v_scr, y_t):
    """Per batch element: attention + proj. Wp resident."""
    with ExitStack() as ctx:
        res = ctx.enter_context(tc.tile_pool(name="p2res", bufs=1))
        io = ctx.enter_context(tc.tile_pool(name="p2io", bufs=1))
        vio = ctx.enter_context(tc.tile_pool(name="p2vio", bufs=2))
        work = ctx.enter_context(tc.tile_pool(name="p2work", bufs=3))
        hb = ctx.enter_context(tc.tile_pool(name="p2hb", bufs=1))
        yio = ctx.enter_context(tc.tile_pool(name="p2yio", bufs=1))
        ps = ctx.enter_context(tc.tile_pool(name="p2ps", bufs=1, space="PSUM"))

        wp_sb = res.tile([128, 24, DIM], F32R)
        bsc_sb = res.tile([128, 12, 2, 256], BF16)   # additive bias, bf16
        pg_sb = res.tile([128, 6], F32)
        pb_sb = res.tile([128, 6], F32)
        six_col = res.tile([128, 1], F32R)           # 6.0 -> rowsum gives 6*sum
        b3 = res.tile([128, 1], F32)
        bm3 = res.tile([128, 1], F32)

        def load_b(b):
            bsl = slice(b * 256, (b + 1) * 256)
            q_b = io.tile([64, 12, 256], F32R, name=f"qb{b}", tag="qb")
            k_b = io.tile([64, 12, 256], F32R, name=f"kb{b}", tag="kb")
            # emission order: q/k heads 0-5, v jt=0, q/k heads 6-11, v jt=1 --
            # S(h=0) starts after the first chunk, AV gets v[0] early
            v_b = [vio.tile([128, DH], F32R, name=f"vb{b}_{jt}", tag="vb")
                   for jt in range(2)]
            for hh in range(2):
                hsl = slice(hh * 6, (hh + 1) * 6)
                nc.sync.dma_start(
                    q_b[:, hsl, :],
                    qk_scr.ap()[hsl, 0:64, bsl].rearrange("h p n -> p h n"))
                nc.sync.dma_start(
                    k_b[:, hsl, :],
                    qk_scr.ap()[hsl, 64:128, bsl].rearrange("h p n -> p h n"))
                nc.sync.dma_start(v_b[hh][:], v_scr.ap()[2 * b + hh])
            return q_b, k_b, v_b

        # issue b=0 loads before the big resident DMAs so they aren't queued
        # behind 12.5MB of weight traffic at the phase boundary
        pre0 = load_b(0)
        nc.sync.dma_start(pg_sb[:], pg.ap())
        nc.sync.dma_start(pb_sb[:], pb.ap())
        nc.sync.dma_start(six_col[:], ones_c.ap())
        nc.vector.memset(b3[:], 3.0)
        nc.vector.memset(bm3[:], -3.0)
        for j in range(2):
            nc.sync.dma_start(bsc_sb[:, :, j, :],
                              bsc.ap()[:, j].rearrange("h p n -> p h n"))
        for dq in range(4):
            nc.sync.dma_start(
                wp_sb[:, dq * 6:(dq + 1) * 6, :],
                wp_t.ap()[dq * 6:(dq + 1) * 6].rearrange("d p c -> p d c"))

        def emit_proj_group(hteff, bprev, ct, yst_prev):
            # proj GEMM group ct of batch elem bprev: y_T[c,:] = sum_d Wp_T.T h_T
            py = ps.tile([128, 256], F32, name=f"py{bprev}_{ct}", tag="py", bufs=2)
            for dt_ in range(24):
                nc.tensor.matmul(
                    py[:], wp_sb[:, dt_, ct * 128:(ct + 1) * 128], hteff[:, dt_, :],
                    start=(dt_ == 0), stop=(dt_ == 23))
            nc.vector.tensor_scalar(
                yst_prev[:, ct, :], py[:], pg_sb[:, ct:ct + 1], pb_sb[:, ct:ct + 1],
                MULT, ADD)
            if ct == 5:
                bslp = slice(bprev * 256, (bprev + 1) * 256)
                nc.sync.dma_start(
                    y_t.ap()[:, :, bslp].rearrange("c p n -> p c n"), yst_prev[:])

        prev = None  # (h_t, b, yst) pending proj, pipelined one b behind
        for b in range(BPC):
            q_b, k_b, v_b = pre0 if b == 0 else load_b(b)
            h_t = hb.tile([128, 24, 256], F32R, name=f"ht{b}", tag="ht", bufs=2)

            for h in range(HEADS):
                # S_T[j, i] for j-tiles of 128; both in one psum bank, jt-major
                s_ps = ps.tile([128, 512], F32, name=f"sps{b}_{h}", tag="sps", bufs=2)
                for jt in range(2):
                    nc.tensor.matmul(
                        s_ps[:, jt * 256:(jt + 1) * 256],
                        k_b[:, h, jt * 128:(jt + 1) * 128],
                        q_b[:, h, :],
                        start=True, stop=True)
                # bias add on DVE (reads PSUM), exp on ACT
                ea = work.tile([128, 512], F32, name=f"ea{b}_{h}", tag="ea", bufs=2)
                nc.vector.tensor_tensor(ea[:], s_ps[:], bsc_sb[:, h], ADD)
                es = work.tile([128, 512], F32R, name=f"es{b}_{h}", tag="es", bufs=2)
                nc.scalar.activation(es[:], ea[:], mybir.ActivationFunctionType.Exp)
                rs_ps = ps.tile([1, 256], F32, name=f"rs{b}_{h}", tag="rs")
                for jt in range(2):
                    nc.tensor.matmul(
                        rs_ps[:], six_col[:], es[:, jt * 256:(jt + 1) * 256],
                        start=(jt == 0), stop=(jt == 1))
                rcp = work.tile([1, 256], F32, name=f"rcp{b}_{h}", tag="rcp", bufs=2)
                nc.vector.reciprocal(rcp[:], rs_ps[:])   # = 1/(6*sum)
                bc_sb = work.tile([128, 256], F32, name=f"bcs{b}_{h}", tag="bcs", bufs=2)
                nc.gpsimd.partition_broadcast(bc_sb[:], rcp[:])
                for dvt in range(2):
                    o_ps = ps.tile([128, 256], F32, name=f"o{b}_{h}_{dvt}", tag="o", bufs=3)
                    for jt in range(2):
                        nc.tensor.matmul(
                            o_ps[:],
                            v_b[jt][:, h * 256 + dvt * 128:h * 256 + (dvt + 1) * 128],
                            es[:, jt * 256:(jt + 1) * 256],
                            start=(jt == 0), stop=(jt == 1))
                    # y0 = O/(6*sum); hardswish(y)=y0*(Relu(6*y0+3)-Relu(6*y0-3))
                    y0 = work.tile([128, 256], F32, name=f"y0{b}_{h}_{dvt}", tag="y0", bufs=3)
                    nc.vector.tensor_tensor(y0[:], o_ps[:], bc_sb[:], MULT)
                    r1 = work.tile([128, 256], F32, name=f"r1{b}_{h}_{dvt}", tag="r1", bufs=2)
                    nc.scalar.activation(r1[:], y0[:], mybir.ActivationFunctionType.Relu,
                                         bias=b3[:], scale=6.0)
                    r2 = work.tile([128, 256], F32, name=f"r2{b}_{h}_{dvt}", tag="r2", bufs=2)
                    nc.scalar.activation(r2[:], y0[:], mybir.ActivationFunctionType.Relu,
                                         bias=bm3[:], scale=6.0)
                    nc.gpsimd.tensor_tensor(r1[:], r1[:], r2[:],
                                            mybir.AluOpType.subtract)
                    nc.vector.tensor_tensor(h_t[:, h * 2 + dvt, :], y0[:], r1[:], MULT)

                # interleave one proj group of the previous b between heads so
                # PE fills the wait for this b's hardswish outputs
                if prev is not None and h % 2 == 1:
                    emit_proj_group(prev[0], prev[1], h // 2, prev[2])

            yst = yio.tile([128, 6, 256], F32, name=f"yst{b}", tag="yst")
            prev = (h_t, b, yst)

        # drain the last b's proj
        for ct in range(6):
            emit_proj_group(prev[0], prev[1], ct, prev[2])


def _build(reps=1, phase="both"):
    nc = bacc.Bacc("TRN2", target_bir_lowering=False, debug=False)
    x_t = nc.dram_tensor("x_t", [6, 128, T], F32R, kind="ExternalInput")
    wqk_t = nc.dram_tensor("wqk_t", [6, 128, 1536], F32R, kind="ExternalInput")
    wv_t = nc.dram_tensor("wv_t", [6, 128, DH], F32R, kind="ExternalInput")
    wp_t = nc.dram_tensor("wp_t", [24, 128, DIM], F32R, kind="ExternalInput")
    bqk = nc.dram_tensor("bqk", [128, 12], F32, kind="ExternalInput")
    bv = nc.dram_tensor("bv", [DH], BF16, kind="ExternalInput")
    bsc = nc.dram_tensor("bsc", [12, 2, 128, 256], BF16, kind="ExternalInput")
    pg = nc.dram_tensor("pg", [128, 6], F32, kind="ExternalInput")
    pb = nc.dram_tensor("pb", [128, 6], F32, kind="ExternalInput")
    y_t = nc.dram_tensor("y_t", [6, 128, T], F32, kind="ExternalOutput")
    ones_c = nc.dram_tensor("ones_c", [128, 1], F32R, kind="ExternalInput")
    qk_scr = nc.dram_tensor("qk_scr", [12, 128, T], F32R)
    v_scr = nc.dram_tensor("v_scr", [16, 128, DH], F32R)

    with tile.TileContext(nc) as tc:
        if reps == 1 and phase == "both":
            _phase1(tc, nc, x_t, wqk_t, wv_t, bqk, bv, qk_scr, v_scr)
            _phase2(tc, nc, wp_t, bsc, pg, pb, ones_c, qk_scr, v_scr, y_t)
        elif phase == "p1":
            with tc.For_i(0, reps, 1):
                _phase1(tc, nc, x_t, wqk_t, wv_t, bqk, bv, qk_scr, v_scr)
            _phase2(tc, nc, wp_t, bsc, pg, pb, ones_c, qk_scr, v_scr, y_t)
        elif phase == "p2":
            _phase1(tc, nc, x_t, wqk_t, wv_t, bqk, bv, qk_scr, v_scr)
            with tc.For_i(0, reps, 1):
                _phase2(tc, nc, wp_t, bsc, pg, pb, ones_c, qk_scr, v_scr, y_t)
        else:
            with tc.For_i(0, reps, 1):
                _phase1(tc, nc, x_t, wqk_t, wv_t, bqk, bv, qk_scr, v_scr)
                _phase2(tc, nc, wp_t, bsc, pg, pb, ones_c, qk_scr, v_scr, y_t)
    nc.compile()
    return nc


_NC = None


def _get_nc():
    global _NC
    if _NC is None:
        _NC = _build()
    return _NC


def _prep_host(qkv_w, qkv_gamma, qkv_beta, qkv_mean, qkv_var,
               attn_biases, proj_w, proj_gamma, proj_beta, proj_mean, proj_var,
               bias_idxs):
    f32 = np.float32
    qkv_w = np.asarray(qkv_w, f32)
    s = np.asarray(qkv_gamma, f32) / np.sqrt(np.asarray(qkv_var, f32) + EPS)
    Wf = qkv_w * s[:, None]
    bf = np.asarray(qkv_beta, f32) - np.asarray(qkv_mean, f32) * s

    base = np.arange(HEADS, dtype=np.int64)[:, None] * 384
    qk_ch = (base + np.arange(128)[None, :]).reshape(-1)
    v_ch = (base + 128 + np.arange(256)[None, :]).reshape(-1)

    Wqk = Wf[qk_ch].copy()
    bqk_v = bf[qk_ch].copy()
    Wqk.reshape(HEADS, 128, DIM)[:, :64, :] *= SCALE
    bqk_v.reshape(HEADS, 128)[:, :64] *= SCALE

    wqk_t = np.ascontiguousarray(Wqk.T).reshape(6, 128, 1536)
    wv_t = np.ascontiguousarray(Wf[v_ch].T).reshape(6, 128, DH)
    bqk_np = np.ascontiguousarray(bqk_v.reshape(HEADS, 128).T)
    bv_np = bf[v_ch].astype(ml_dtypes.bfloat16)

    proj_w = np.asarray(proj_w, f32)
    sp = np.asarray(proj_gamma, f32) / np.sqrt(np.asarray(proj_var, f32) + EPS)
    bp_v = np.asarray(proj_beta, f32) - np.asarray(proj_mean, f32) * sp
    wp_t = np.ascontiguousarray(proj_w.T).reshape(24, 128, DIM)
    pg_np = np.ascontiguousarray(sp.reshape(6, 128).T)
    pb_np = np.ascontiguousarray(bp_v.reshape(6, 128).T)

    bias_full = np.asarray(attn_biases, f32)[:, np.asarray(bias_idxs)]  # [h, i, j]
    bsc_np = np.ascontiguousarray(
        bias_full.transpose(0, 2, 1)).reshape(HEADS, 2, 128, 256).astype(
        ml_dtypes.bfloat16)

    return dict(wqk_t=wqk_t, wv_t=wv_t, bqk=bqk_np, bv=bv_np,
                wp_t=wp_t, pg=pg_np, pb=pb_np, bsc=bsc_np,
                ones_c=np.full((128, 1), 6.0, f32))


def kernel(x, qkv_w, qkv_gamma, qkv_beta, qkv_mean, qkv_var,
           attn_biases, proj_w, proj_gamma, proj_beta, proj_mean, proj_var,
           bias_idxs):
    x = np.asarray(x, np.float32)
    shared = _prep_host(qkv_w, qkv_gamma, qkv_beta, qkv_mean, qkv_var,
                        attn_biases, proj_w, proj_gamma, proj_beta,
                        proj_mean, proj_var, bias_idxs)
    in_maps = []
    for ci in range(NCORES):
        xc = x[ci * BPC:(ci + 1) * BPC].reshape(T, DIM)
        x_tc = np.ascontiguousarray(xc.T).reshape(6, 128, T)
        m = dict(shared)
        m["x_t"] = x_tc
        in_maps.append(m)

    nc = _get_nc()
    res = run_bass_kernel_spmd(nc, in_maps, core_ids=list(range(NCORES)))

    out = np.empty((B, SEQ, DIM), np.float32)
    for ci in range(NCORES):
        yt = np.asarray(res.results[ci]["y_t"]).reshape(DIM, T)
        out[ci * BPC:(ci + 1) * BPC] = yt.T.reshape(BPC, SEQ, DIM)
    return out


# revision 36
# speedup vs baseline: 1.9067x; 1.9067x over previous
"""Trainium2 Bass kernel for nn_Attention_46901042872659.

Dense transformer attention block:
  qkv = BN(x @ qkv_w.T); split q,k,v per head; attn = softmax(q k^T * scale + bias);
  out = hardswish(attn @ v); y = BN(out @ proj_w.T)

Strategy: data-parallel over batch across 8 NeuronCores (8 batch elems each).
All matmuls in fp32r (full PE rate at moving-dim>=256, ~1e-4 accuracy).
Eval-mode BN is folded into the GEMM weights/bias on the host; the softmax
scale is folded into the q weights; the relative-position bias table is
gathered host-side into a dense [heads, j, i] matrix (constant wrt x).

Device program per core (single compiled module, run SPMD on 8 cores):
  Phase 1: qkv GEMM over all 2048 tokens.
    - q,k produced channel-major (q_T/k_T [64, tokens] per head) -> qk_scr DRAM
    - v produced token-major ([tokens, 3072]) -> v_scr DRAM
  Phase 2: per batch element b (8 of them):
    - S_T[j,i] = k_T^T q_T (contraction d=64), + bias (DVE), exp (ACT)
    - rowsum over j via ones-matmul; reciprocal; broadcast via rank-1 matmul
    - O_T[dv,i] = v^T @ expS_T (contraction j=256)
    - hardswish(O_T * recip) -> h_T (channel-major) feeds proj GEMM directly
    - y_T = Wp_T^T @ h_T (contraction d=3072) + BN fold -> out
"""
import numpy as np
import ml_dtypes
from contextlib import ExitStack

import concourse.bass as bass
import concourse.tile as tile
from concourse import bacc, mybir
from concourse.bass_utils import run_bass_kernel_spmd

# problem constants (hardcoded per contest contract)
B, SEQ, DIM = 64, 256, 768
HEADS, KD, DV = 12, 64, 256
H = 4608
DH = 3072
EPS = 1e-5
SCALE = KD ** -0.5
NCORES = 8
BPC = B // NCORES          # batch elems per core
T = BPC * SEQ              # tokens per core = 2048
F32 = mybir.dt.float32
F32R = mybir.dt.float32r
BF16 = mybir.dt.bfloat16
ADD = mybir.AluOpType.add
MULT = mybir.AluOpType.mult
MAX = mybir.AluOpType.max
MIN = mybir.AluOpType.min


def _phase1(tc, nc, x_t, wqk_t, wv_t, bqk, bv, qk_scr, v_scr):
    """qkv GEMM for all T tokens. x_T resident; weights resident."""
    with ExitStack() as ctx:
        res = ctx.enter_context(tc.tile_pool(name="p1res", bufs=1))
        st = ctx.enter_context(tc.tile_pool(name="p1st", bufs=2))
        ps = ctx.enter_context(tc.tile_pool(name="p1ps", bufs=1, space="PSUM"))

        x_sb = res.tile([128, 6, T], F32R)
        wqk_sb = res.tile([128, 6, 1536], F32R)
        wv_sb = res.tile([128, 6, DH], F32R)
        bqk_sb = res.tile([128, 12], F32)
        bvb_sb = res.tile([128, DH], BF16)
        for c in range(6):
            nc.sync.dma_start(x_sb[:, c, :], x_t.ap()[c])
            nc.sync.dma_start(wqk_sb[:, c, :], wqk_t.ap()[c])
        for c in range(6):
            nc.sync.dma_start(wv_sb[:, c, :], wv_t.ap()[c])
        nc.sync.dma_start(bqk_sb[:], bqk.ap())
        bv_ap = bv.ap()
        bv_bcast = bass.AP(tensor=bv_ap.tensor, offset=bv_ap.offset,
                           ap=[[0, 128]] + [list(p) for p in bv_ap.ap])
        nc.gpsimd.dma_start(bvb_sb[:], bv_bcast)

        # q,k part: out channel-major [co 128, t], lhsT = Wqk tile, rhs = x_T
        for cot in range(12):
            qkst = st.tile([128, T], F32R, name=f"qkst{cot}", tag="qkst")
            for tb in range(4):
                qps = ps.tile([128, 512], F32, name=f"qps{cot}_{tb}", tag="qps",
                              bufs=2)
                for c in range(6):
                    nc.tensor.matmul(
                        qps[:],
                        wqk_sb[:, c, cot * 128:(cot + 1) * 128],
                        x_sb[:, c, tb * 512:(tb + 1) * 512],
                        start=(c == 0), stop=(c == 5))
                nc.vector.tensor_scalar_add(
                    qkst[:, tb * 512:(tb + 1) * 512], qps[:], bqk_sb[:, cot:cot + 1])
            nc.sync.dma_start(qk_scr.ap()[cot], qkst[:])

        # v part: out token-major [t 128, co], lhsT = x_T tile (stationary), rhs = Wv
        for tt in range(16):
            vst = st.tile([128, DH], F32R, name=f"vst{tt}", tag="vst")
            vps_l = [ps.tile([128, 512], F32, name=f"vps{tt}_{cob}", tag=f"vps{cob}")
                     for cob in range(6)]
            for c in range(6):
                for cob in range(6):
                    nc.tensor.matmul(
                        vps_l[cob][:],
                        x_sb[:, c, tt * 128:(tt + 1) * 128],
                        wv_sb[:, c, cob * 512:(cob + 1) * 512],
                        start=(c == 0), stop=(c == 5))
            for cob in range(6):
                nc.vector.tensor_tensor(
                    vst[:, cob * 512:(cob + 1) * 512], vps_l[cob][:],
                    bvb_sb[:, cob * 512:(cob + 1) * 512], ADD)
            nc.sync.dma_start(v_scr.ap()[tt], vst[:])


def _phase2(tc, nc, wp_t, bsc, pg, pb, ones_c, qk_scr, v_scr, y_t):
    """Per batch element: attention + proj. Wp resident."""
    with ExitStack() as ctx:
        res = ctx.enter_context(tc.tile_pool(name="p2res", bufs=1))
        io = ctx.enter_context(tc.tile_pool(name="p2io", bufs=1))
        vio = ctx.enter_context(tc.tile_pool(name="p2vio", bufs=2))
        work = ctx.enter_context(tc.tile_pool(name="p2work", bufs=3))
        hb = ctx.enter_context(tc.tile_pool(name="p2hb", bufs=1))
        yio = ctx.enter_context(tc.tile_pool(name="p2yio", bufs=1))
        ps = ctx.enter_context(tc.tile_pool(name="p2ps", bufs=1, space="PSUM"))

        wp_sb = res.tile([128, 24, DIM], F32R)
        bsc_sb = res.tile([128, 12, 2, 256], BF16)   # additive bias, bf16
        pg_sb = res.tile([128, 6], F32)
        pb_sb = res.tile([128, 6], F32)
        six_col = res.tile([128, 1], F32R)           # 6.0 -> rowsum gives 6*sum
        b3 = res.tile([128, 1], F32)
        bm3 = res.tile([128, 1], F32)

        def load_b(b):
            # q/k/v split into head-half tiles (heads 0-5 / 6-11) so each
            # half's slot frees mid-b and b+1's prefetch hides under heads 6-11
            bsl = slice(b * 256, (b + 1) * 256)
            q_b, k_b, v_b = [], [], [[], []]
            for hh in range(2):
                hsl = slice(hh * 6, (hh + 1) * 6)
                csl = slice(hh * 1536, (hh + 1) * 1536)
                qt = io.tile([64, 6, 256], F32R, name=f"qb{b}_{hh}", tag=f"qb{hh}")
                kt = io.tile([64, 6, 256], F32R, name=f"kb{b}_{hh}", tag=f"kb{hh}")
                nc.sync.dma_start(
                    qt[:], qk_scr.ap()[hsl, 0:64, bsl].rearrange("h p n -> p h n"))
                nc.sync.dma_start(
                    kt[:], qk_scr.ap()[hsl, 64:128, bsl].rearrange("h p n -> p h n"))
                q_b.append(qt)
                k_b.append(kt)
                for jt in range(2):
                    vt = vio.tile([128, 1536], F32R, name=f"vb{b}_{jt}_{hh}",
                                  tag=f"vb{jt}{hh}", bufs=1)
                    nc.sync.dma_start(vt[:], v_scr.ap()[2 * b + jt][:, csl])
                    v_b[jt].append(vt)
            return q_b, k_b, v_b

        # issue b=0 loads before the big resident DMAs so they aren't queued
        # behind 12.5MB of weight traffic at the phase boundary
        pre0 = load_b(0)
        nc.sync.dma_start(pg_sb[:], pg.ap())
        nc.sync.dma_start(pb_sb[:], pb.ap())
        nc.sync.dma_start(six_col[:], ones_c.ap())
        nc.vector.memset(b3[:], 3.0)
        nc.vector.memset(bm3[:], -3.0)
        for j in range(2):
            nc.sync.dma_start(bsc_sb[:, :, j, :],
                              bsc.ap()[:, j].rearrange("h p n -> p h n"))
        for dq in range(4):
            nc.sync.dma_start(
                wp_sb[:, dq * 6:(dq + 1) * 6, :],
                wp_t.ap()[dq * 6:(dq + 1) * 6].rearrange("d p c -> p d c"))

        def emit_proj_group(hteff, bprev, ct, yst_prev):
            # proj GEMM group ct of batch elem bprev: y_T[c,:] = sum_d Wp_T.T h_T
            py = ps.tile([128, 256], F32, name=f"py{bprev}_{ct}", tag="py", bufs=2)
            for dt_ in range(24):
                nc.tensor.matmul(
                    py[:], wp_sb[:, dt_, ct * 128:(ct + 1) * 128], hteff[:, dt_, :],
                    start=(dt_ == 0), stop=(dt_ == 23))
            nc.vector.tensor_scalar(
                yst_prev[:, ct, :], py[:], pg_sb[:, ct:ct + 1], pb_sb[:, ct:ct + 1],
                MULT, ADD)
            if ct == 5:
                bslp = slice(bprev * 256, (bprev + 1) * 256)
                nc.sync.dma_start(
                    y_t.ap()[:, :, bslp].rearrange("c p n -> p c n"), yst_prev[:])

        prev = None  # (h_t, b, yst) pending proj, pipelined one b behind
        for b in range(BPC):
            q_b, k_b, v_b = pre0 if b == 0 else load_b(b)
            h_t = hb.tile([128, 24, 256], F32R, name=f"ht{b}", tag="ht", bufs=2)

            for h in range(HEADS):
                hh, hi_ = h // 6, h % 6
                # S_T[j, i] for j-tiles of 128; both in one psum bank, jt-major
                s_ps = ps.tile([128, 512], F32, name=f"sps{b}_{h}", tag="sps", bufs=2)
                for jt in range(2):
                    nc.tensor.matmul(
                        s_ps[:, jt * 256:(jt + 1) * 256],
                        k_b[hh][:, hi_, jt * 128:(jt + 1) * 128],
                        q_b[hh][:, hi_, :],
                        start=True, stop=True)
                # bias add on DVE (reads PSUM), exp on ACT
                ea = work.tile([128, 512], F32, name=f"ea{b}_{h}", tag="ea", bufs=2)
                nc.vector.tensor_tensor(ea[:], s_ps[:], bsc_sb[:, h], ADD)
                es = work.tile([128, 512], F32R, name=f"es{b}_{h}", tag="es", bufs=2)
                nc.scalar.activation(es[:], ea[:], mybir.ActivationFunctionType.Exp)
                rs_ps = ps.tile([1, 256], F32, name=f"rs{b}_{h}", tag="rs")
                for jt in range(2):
                    nc.tensor.matmul(
                        rs_ps[:], six_col[:], es[:, jt * 256:(jt + 1) * 256],
                        start=(jt == 0), stop=(jt == 1))
                rcp = work.tile([1, 256], F32, name=f"rcp{b}_{h}", tag="rcp", bufs=2)
                nc.vector.reciprocal(rcp[:], rs_ps[:])   # = 1/(6*sum)
                bc_sb = work.tile([128, 256], F32, name=f"bcs{b}_{h}", tag="bcs", bufs=2)
                nc.gpsimd.partition_broadcast(bc_sb[:], rcp[:])
                for dvt in range(2):
                    o_ps = ps.tile([128, 256], F32, name=f"o{b}_{h}_{dvt}", tag="o", bufs=3)
                    for jt in range(2):
                        nc.tensor.matmul(
                            o_ps[:],
                            v_b[jt][hh][:, hi_ * 256 + dvt * 128:
                                        hi_ * 256 + (dvt + 1) * 128],
                            es[:, jt * 256:(jt + 1) * 256],
                            start=(jt == 0), stop=(jt == 1))
                    # y0 = O/(6*sum); hardswish(y)=y0*(Relu(6*y0+3)-Relu(6*y0-3))
                    y0 = work.tile([128, 256], F32, name=f"y0{b}_{h}_{dvt}", tag="y0", bufs=3)
                    nc.vector.tensor_tensor(y0[:], o_ps[:], bc_sb[:], MULT)
                    r1 = work.tile([128, 256], F32, name=f"r1{b}_{h}_{dvt}", tag="r1", bufs=2)
                    nc.scalar.activation(r1[:], y0[:], mybir.ActivationFunctionType.Relu,
                                         bias=b3[:], scale=6.0)
                    r2 = work.tile([128, 256], F32, name=f"r2{b}_{h}_{dvt}", tag="r2", bufs=2)
                    nc.scalar.activation(r2[:], y0[:], mybir.ActivationFunctionType.Relu,
                                         bias=bm3[:], scale=6.0)
                    nc.gpsimd.tensor_tensor(r1[:], r1[:], r2[:],
                                            mybir.AluOpType.subtract)
                    nc.vector.tensor_tensor(h_t[:, h * 2 + dvt, :], y0[:], r1[:], MULT)

                # interleave one proj group of the previous b between heads so
                # PE fills the wait for this b's hardswish outputs
                if prev is not None and h % 2 == 1:
                    emit_proj_group(prev[0], prev[1], h // 2, prev[2])

            yst = yio.tile([128, 6, 256], F32, name=f"yst{b}", tag="yst")
            prev = (h_t, b, yst)

        # drain the last b's proj
        for ct in range(6):
            emit_proj_group(prev[0], prev[1], ct, prev[2])


def _build(reps=1, phase="both"):
    nc = bacc.Bacc("TRN2", target_bir_lowering=False, debug=False)
    x_t = nc.dram_tensor("x_t", [6, 128, T], F32R, kind="ExternalInput")
    wqk_t = nc.dram_tensor("wqk_t", [6, 128, 1536], F32R, kind="ExternalInput")
    wv_t = nc.dram_tensor("wv_t", [6, 128, DH], F32R, kind="ExternalInput")
    wp_t = nc.dram_tensor("wp_t", [24, 128, DIM], F32R, kind="ExternalInput")
    bqk = nc.dram_tensor("bqk", [128, 12], F32, kind="ExternalInput")
    bv = nc.dram_tensor("bv", [DH], BF16, kind="ExternalInput")
    bsc = nc.dram_tensor("bsc", [12, 2, 128, 256], BF16, kind="ExternalInput")
    pg = nc.dram_tensor("pg", [128, 6], F32, kind="ExternalInput")
    pb = nc.dram_tensor("pb", [128, 6], F32, kind="ExternalInput")
    y_t = nc.dram_tensor("y_t", [6, 128, T], F32, kind="ExternalOutput")
    ones_c = nc.dram_tensor("ones_c", [128, 1], F32R, kind="ExternalInput")
    qk_scr = nc.dram_tensor("qk_scr", [12, 128, T], F32R)
    v_scr = nc.dram_tensor("v_scr", [16, 128, DH], F32R)

    with tile.TileContext(nc) as tc:
        if reps == 1 and phase == "both":
            _phase1(tc, nc, x_t, wqk_t, wv_t, bqk, bv, qk_scr, v_scr)
            _phase2(tc, nc, wp_t, bsc, pg, pb, ones_c, qk_scr, v_scr, y_t)
        elif phase == "p1":
            with tc.For_i(0, reps, 1):
                _phase1(tc, nc, x_t, wqk_t, wv_t, bqk, bv, qk_scr, v_scr)
            _phase2(tc, nc, wp_t, bsc, pg, pb, ones_c, qk_scr, v_scr, y_t)
        elif phase == "p2":
            _phase1(tc, nc, x_t, wqk_t, wv_t, bqk, bv, qk_scr, v_scr)
            with tc.For_i(0, reps, 1):
                _phase2(tc, nc, wp_t, bsc, pg, pb, ones_c, qk_scr, v_scr, y_t)
        else:
            with tc.For_i(0, reps, 1):
                _phase1(tc, nc, x_t, wqk_t, wv_t, bqk, bv, qk_scr, v_scr)
                _phase2(tc, nc, wp_t, bsc, pg, pb, ones_c, qk_scr, v_scr, y_t)
    nc.compile()
    return nc


_NC = None


def _get_nc():
    global _NC
    if _NC is None:
        _NC = _build()
    return _NC


def _prep_host(qkv_w, qkv_gamma, qkv_beta, qkv_mean, qkv_var,
               attn_biases, proj_w, proj_gamma, proj_beta, proj_mean, proj_var,
               bias_idxs):
    f32 = np.float32
    qkv_w = np.asarray(qkv_w, f32)
    s = np.asarray(qkv_gamma, f32) / np.sqrt(np.asarray(qkv_var, f32) + EPS)
    Wf = qkv_w * s[:, None]
    bf = np.asarray(qkv_beta, f32) - np.asarray(qkv_mean, f32) * s

    base = np.arange(HEADS, dtype=np.int64)[:, None] * 384
    qk_ch = (base + np.arange(128)[None, :]).reshape(-1)
    v_ch = (base + 128 + np.arange(256)[None, :]).reshape(-1)

    Wqk = Wf[qk_ch].copy()
    bqk_v = bf[qk_ch].copy()
    Wqk.reshape(HEADS, 128, DIM)[:, :64, :] *= SCALE
    bqk_v.reshape(HEADS, 128)[:, :64] *= SCALE

    wqk_t = np.ascontiguousarray(Wqk.T).reshape(6, 128, 1536)
    wv_t = np.ascontiguousarray(Wf[v_ch].T).reshape(6, 128, DH)
    bqk_np = np.ascontiguousarray(bqk_v.reshape(HEADS, 128).T)
    bv_np = bf[v_ch].astype(ml_dtypes.bfloat16)

    proj_w = np.asarray(proj_w, f32)
    sp = np.asarray(proj_gamma, f32) / np.sqrt(np.asarray(proj_var, f32) + EPS)
    bp_v = np.asarray(proj_beta, f32) - np.asarray(proj_mean, f32) * sp
    wp_t = np.ascontiguousarray(proj_w.T).reshape(24, 128, DIM)
    pg_np = np.ascontiguousarray(sp.reshape(6, 128).T)
    pb_np = np.ascontiguousarray(bp_v.reshape(6, 128).T)

    bias_full = np.asarray(attn_biases, f32)[:, np.asarray(bias_idxs)]  # [h, i, j]
    bsc_np = np.ascontiguousarray(
        bias_full.transpose(0, 2, 1)).reshape(HEADS, 2, 128, 256).astype(
        ml_dtypes.bfloat16)

    return dict(wqk_t=wqk_t, wv_t=wv_t, bqk=bqk_np, bv=bv_np,
                wp_t=wp_t, pg=pg_np, pb=pb_np, bsc=bsc_np,
                ones_c=np.full((128, 1), 6.0, f32))


def kernel(x, qkv_w, qkv_gamma, qkv_beta, qkv_mean, qkv_var,
           attn_biases, proj_w, proj_gamma, proj_beta, proj_mean, proj_var,
           bias_idxs):
    x = np.asarray(x, np.float32)
    shared = _prep_host(qkv_w, qkv_gamma, qkv_beta, qkv_mean, qkv_var,
                        attn_biases, proj_w, proj_gamma, proj_beta,
                        proj_mean, proj_var, bias_idxs)
    in_maps = []
    for ci in range(NCORES):
        xc = x[ci * BPC:(ci + 1) * BPC].reshape(T, DIM)
        x_tc = np.ascontiguousarray(xc.T).reshape(6, 128, T)
        m = dict(shared)
        m["x_t"] = x_tc
        in_maps.append(m)

    nc = _get_nc()
    res = run_bass_kernel_spmd(nc, in_maps, core_ids=list(range(NCORES)))

    out = np.empty((B, SEQ, DIM), np.float32)
    for ci in range(NCORES):
        yt = np.asarray(res.results[ci]["y_t"]).reshape(DIM, T)
        out[ci * BPC:(ci + 1) * BPC] = yt.T.reshape(BPC, SEQ, DIM)
    return out
